# revision 4
# baseline (speedup 1.0000x reference)
"""Trainium2 Bass kernel for nn_Encoder_38697655337457.

The reference module (due to its never-advancing `start` index) computes a
single LSTM-cell step (zero initial state) on the t=0 token embeddings, then
broadcasts the top-layer h across time under a length mask.  Consequences:
  * h0 == 0 for every layer, so W_hh never contributes and the f-gate is dead.
  * The pack/sort permutation cancels for `output`; it only selects which rows
    (the length==S ones, in stable order) land in state_h/state_c.

Strategy (8 NeuronCores, no collectives):
  * Replicate the embedding table and the (fp16-cast, i/g/o-only, transposed)
    W_ih on every core; each core runs the full 4-layer single-step chain.
  * Core c writes output[64c:64c+64] (t-sharded, contiguous 512KB rows) and
    the 16 state slots [16c:16c+16] of state_h/state_c.
  * Host concatenates the per-core chunks.
"""
import os
import sys

sys.path.insert(0, "/opt/trn_rl_repo")

import numpy as np
import concourse.bass as bass
import concourse.bacc as bacc
import concourse.tile as tile
import concourse.mybir as mybir
from concourse import bass_utils

B, S, H, L, V = 128, 512, 1024, 4, 32000
N_CORES = 8
TCHUNK = S // N_CORES  # 64 timesteps per core
JSLOT = B // N_CORES  # 16 state slots per core
G3 = 3 * H  # i|g|o packed gate width
KC = H // 128  # 8 contraction chunks
TS_PER_BUF = 2  # timesteps per output staging buffer (1MB DMAs)

F32 = mybir.dt.float32
F16 = mybir.dt.float16
I32 = mybir.dt.int32


def _build():
    nc = bacc.Bacc("TRN2", target_bir_lowering=False, debug=False, num_devices=N_CORES)

    x32 = nc.dram_tensor("x32", [B, S], I32, kind="ExternalInput").ap()
    emb = nc.dram_tensor("emb", [V, H], F32, kind="ExternalInput").ap()
    w = nc.dram_tensor("w", [L, KC, 128, G3], F16, kind="ExternalInput").ap()
    bias = nc.dram_tensor("bias", [1, L * G3], F16, kind="ExternalInput").ap()
    ident = nc.dram_tensor("ident", [128, 128], F32, kind="ExternalInput").ap()
    ltri = nc.dram_tensor("ltri", [128, 128], F32, kind="ExternalInput").ap()
    iota_t = nc.dram_tensor("iota_t", [128, TCHUNK], F32, kind="ExternalInput").ap()
    iota_j = nc.dram_tensor("iota_j", [128, JSLOT], F32, kind="ExternalInput").ap()

    out_chunk = nc.dram_tensor("out_chunk", [TCHUNK, B, H], F32, kind="ExternalOutput").ap()
    st_h = nc.dram_tensor("st_h", [L, JSLOT, H], F32, kind="ExternalOutput").ap()
    st_c = nc.dram_tensor("st_c", [L, JSLOT, H], F32, kind="ExternalOutput").ap()

    with tile.TileContext(nc) as tc:
        with (
            tc.tile_pool(name="consts", bufs=1) as cpool,
            tc.tile_pool(name="work", bufs=1) as wk,
            tc.tile_pool(name="weights", bufs=2) as wpool,
            tc.tile_pool(name="xtT", bufs=2) as xpool,
            tc.tile_pool(name="obuf", bufs=2) as opool,
            tc.tile_pool(name="psum", bufs=1, space="PSUM") as pp,
            tc.tile_pool(name="psum_tr", bufs=2, space="PSUM") as ptr,
        ):
            # ---- constants -------------------------------------------------
            ident_sb = cpool.tile([128, 128], F32)
            nc.sync.dma_start(out=ident_sb[:], in_=ident[:])
            ltri_sb = cpool.tile([128, 128], F32)
            nc.sync.dma_start(out=ltri_sb[:], in_=ltri[:])
            iota_t_sb = cpool.tile([128, TCHUNK], F32)
            nc.sync.dma_start(out=iota_t_sb[:], in_=iota_t[:])
            iota_j_sb = cpool.tile([128, JSLOT], F32)
            nc.sync.dma_start(out=iota_j_sb[:], in_=iota_j[:])
            bias_sb = cpool.tile([1, L * G3], F16)
            nc.sync.dma_start(out=bias_sb[:], in_=bias[:])
            ones_sb = cpool.tile([1, 128], F16)
            nc.vector.memset(ones_sb[:], 1.0)

            # ---- lengths ---------------------------------------------------
            sx = wk.tile([B, S], I32)
            nc.sync.dma_start(out=sx[:], in_=x32[:])
            sxf = wk.tile([B, S], F32)
            nc.vector.tensor_copy(sxf[:], sx[:])
            nc.vector.tensor_scalar(
                out=sxf[:], in0=sxf[:], scalar1=0.0, scalar2=None,
                op0=mybir.AluOpType.is_gt,
            )
            lenf = wk.tile([B, 1], F32)
            nc.vector.reduce_sum(out=lenf[:], in_=sxf[:], axis=mybir.AxisListType.X)

            # ---- embedding gather (t=0 tokens) -----------------------------
            xt0 = wk.tile([B, H], F32)
            nc.gpsimd.indirect_dma_start(
                out=xt0[:],
                out_offset=None,
                in_=emb[:],
                in_offset=bass.IndirectOffsetOnAxis(ap=sx[:, 0:1], axis=0),
            )

            def transpose_to_f16(src_f32, tag):
                """[128,1024] f32 -> fp16 transposed chunks [128, KC*128]."""
                dst = xpool.tile([128, KC * 128], F16, tag=tag)
                for kc in range(KC):
                    tp = ptr.tile([128, 128], F32, tag="tr")
                    nc.tensor.transpose(
                        tp[:], src_f32[:, kc * 128 : (kc + 1) * 128], ident_sb[:]
                    )
                    nc.vector.tensor_copy(dst[:, kc * 128 : (kc + 1) * 128], tp[:])
                return dst

            xtT = transpose_to_f16(xt0, "xtT")

            # ---- 4-layer single-step LSTM chain ----------------------------
            h_all = wk.tile([B, L * H], F32)
            c_all = wk.tile([B, L * H], F32)
            for l in range(L):
                wt = wpool.tile([128, KC * G3], F16, tag="w")
                for kc in range(KC):
                    nc.sync.dma_start(
                        out=wt[:, kc * G3 : (kc + 1) * G3], in_=w[l, kc]
                    )
                gates = pp.tile([128, G3], F32, tag="gates")
                n_slices = G3 // 512
                for n in range(n_slices):
                    nc.tensor.matmul(
                        gates[:, n * 512 : (n + 1) * 512],
                        lhsT=ones_sb[:],
                        rhs=bias_sb[0:1, l * G3 + n * 512 : l * G3 + (n + 1) * 512],
                        start=True,
                        stop=False,
                    )
                for kc in range(KC):
                    for n in range(n_slices):
                        nc.tensor.matmul(
                            gates[:, n * 512 : (n + 1) * 512],
                            lhsT=xtT[:, kc * 128 : (kc + 1) * 128],
                            rhs=wt[:, kc * G3 + n * 512 : kc * G3 + (n + 1) * 512],
                            start=False,
                            stop=(kc == KC - 1),
                        )
                sig_i = wk.tile([B, H], F32, tag="sig_i")
                nc.scalar.activation(
                    sig_i[:], gates[:, 0:H], mybir.ActivationFunctionType.Sigmoid
                )
                tanh_g = wk.tile([B, H], F32, tag="tanh_g")
                nc.scalar.activation(
                    tanh_g[:], gates[:, H : 2 * H], mybir.ActivationFunctionType.Tanh
                )
                c_l = c_all[:, l * H : (l + 1) * H]
                nc.vector.tensor_tensor(
                    out=c_l, in0=sig_i[:], in1=tanh_g[:], op=mybir.AluOpType.mult
                )
                tanh_c = wk.tile([B, H], F32, tag="sig_i")
                nc.scalar.activation(
                    tanh_c[:], c_l, mybir.ActivationFunctionType.Tanh
                )
                sig_o = wk.tile([B, H], F32, tag="tanh_g")
                nc.scalar.activation(
                    sig_o[:], gates[:, 2 * H : 3 * H], mybir.ActivationFunctionType.Sigmoid
                )
                h_l = h_all[:, l * H : (l + 1) * H]
                nc.vector.tensor_tensor(
                    out=h_l, in0=sig_o[:], in1=tanh_c[:], op=mybir.AluOpType.mult
                )
                if l < L - 1:
                    xtT = transpose_to_f16(h_l, "xtT")

            # ---- output: per-t masked broadcast of h4 ----------------------
            mask = wk.tile([B, TCHUNK], F32)
            nc.vector.tensor_tensor(
                out=mask[:],
                in0=iota_t_sb[:],
                in1=lenf[:].to_broadcast([B, TCHUNK]),
                op=mybir.AluOpType.is_lt,
            )
            h4 = h_all[:, (L - 1) * H : L * H]
            for g in range(TCHUNK // TS_PER_BUF):
                buf = opool.tile([128, TS_PER_BUF * H], F32, tag="obuf")
                for j in range(TS_PER_BUF):
                    t = g * TS_PER_BUF + j
                    nc.vector.tensor_tensor(
                        out=buf[:, j * H : (j + 1) * H],
                        in0=h4,
                        in1=mask[:, t : t + 1].to_broadcast([B, H]),
                        op=mybir.AluOpType.mult,
                    )
                dst = out_chunk[g * TS_PER_BUF : (g + 1) * TS_PER_BUF].rearrange(
                    "t b h -> b t h"
                )
                nc.sync.dma_start(out=dst, in_=buf[:])

            # ---- states: select rows with length==S in stable order --------
            e_sel = wk.tile([B, 1], F32)
            nc.vector.tensor_scalar(
                out=e_sel[:], in0=lenf[:], scalar1=float(S), scalar2=None,
                op0=mybir.AluOpType.is_equal,
            )
            r_ps = ptr.tile([128, 1], F32, tag="tr")
            nc.tensor.matmul(r_ps[:], lhsT=ltri_sb[:], rhs=e_sel[:], start=True, stop=True)
            rank = wk.tile([B, 1], F32)
            nc.vector.tensor_scalar(
                out=rank[:], in0=r_ps[:], scalar1=-1.0, scalar2=None,
                op0=mybir.AluOpType.add,
            )
            pt = wk.tile([B, JSLOT], F32)
            nc.vector.tensor_tensor(
                out=pt[:], in0=iota_j_sb[:], in1=rank[:].to_broadcast([B, JSLOT]),
                op=mybir.AluOpType.is_equal,
            )
            nc.vector.tensor_tensor(
                out=pt[:], in0=pt[:], in1=e_sel[:].to_broadcast([B, JSLOT]),
                op=mybir.AluOpType.mult,
            )
            for l in range(L):
                for src_all, dst in ((h_all, st_h), (c_all, st_c)):
                    sp = pp.tile([JSLOT, H], F32, tag="gates")
                    for n in range(H // 512):
                        nc.tensor.matmul(
                            sp[:, n * 512 : (n + 1) * 512],
                            lhsT=pt[:],
                            rhs=src_all[:, l * H + n * 512 : l * H + (n + 1) * 512],
                            start=True,
                            stop=True,
                        )
                    stile = wk.tile([JSLOT, H], F32, tag="stile")
                    nc.vector.tensor_copy(stile[:], sp[:])
                    nc.sync.dma_start(out=dst[l], in_=stile[:])

    nc.compile()
    return nc


_NC_CACHE = {}


def _get_nc():
    if "nc" not in _NC_CACHE:
        _NC_CACHE["nc"] = _build()
    return _NC_CACHE["nc"]


def _prep_in_maps(x, emb, W_ih, b_ih, b_hh):
    x32 = np.ascontiguousarray(np.asarray(x).astype(np.int32))
    emb = np.ascontiguousarray(np.asarray(emb, dtype=np.float32))
    W_ih = np.asarray(W_ih, dtype=np.float32)
    b_ih = np.asarray(b_ih, dtype=np.float32)
    b_hh = np.asarray(b_hh, dtype=np.float32)

    igo = np.r_[0:H, 2 * H : 3 * H, 3 * H : 4 * H]
    # rhs layout: [l, kc, k_part, j]  (j = i|g|o packed), fp16
    w_t = np.ascontiguousarray(
        W_ih[:, igo, :].transpose(0, 2, 1).reshape(L, KC, 128, G3).astype(np.float16)
    )
    bias = np.ascontiguousarray((b_ih + b_hh)[:, igo].astype(np.float16).reshape(1, L * G3))
    ident = np.eye(128, dtype=np.float32)
    ltri = (np.arange(128)[:, None] <= np.arange(128)[None, :]).astype(np.float32)

    in_maps = []
    for c in range(N_CORES):
        iota_t = np.broadcast_to(
            (TCHUNK * c + np.arange(TCHUNK)).astype(np.float32), (128, TCHUNK)
        ).copy()
        iota_j = np.broadcast_to(
            (JSLOT * c + np.arange(JSLOT)).astype(np.float32), (128, JSLOT)
        ).copy()
        in_maps.append(
            {
                "x32": x32,
                "emb": emb,
                "w": w_t,
                "bias": bias,
                "ident": ident,
                "ltri": ltri,
                "iota_t": iota_t,
                "iota_j": iota_j,
            }
        )
    return in_maps


def kernel(x, emb, W_ih, W_hh, b_ih, b_hh):
    nc = _get_nc()
    in_maps = _prep_in_maps(x, emb, W_ih, b_ih, b_hh)
    trace = bool(os.environ.get("BASS_KERNEL_TRACE"))
    res = bass_utils.run_bass_kernel_spmd(
        nc, in_maps, core_ids=list(range(N_CORES)), trace=trace
    )
    if trace and res.exec_time_ns is not None:
        print(f"HW exec time: {res.exec_time_ns} ns")
    output = np.concatenate(
        [res.results[c]["out_chunk"] for c in range(N_CORES)], axis=0
    )
    state_h = np.concatenate(
        [res.results[c]["st_h"] for c in range(N_CORES)], axis=1
    )
    state_c = np.concatenate(
        [res.results[c]["st_c"] for c in range(N_CORES)], axis=1
    )
    return output, state_h, state_c


# revision 10
# speedup vs baseline: 1.0668x; 1.0668x over previous
"""Trainium2 Bass kernel for nn_Encoder_38697655337457.

The reference module (due to its never-advancing `start` index) computes a
single LSTM-cell step (zero initial state) on the t=0 token embeddings, then
broadcasts the top-layer h across time under a length mask.  Consequences:
  * h0 == 0 for every layer, so W_hh never contributes and the f-gate is dead.
  * The pack/sort permutation cancels for `output`; it only selects which rows
    (the length==S ones, in stable order) land in state_h/state_c.

Strategy (8 NeuronCores, no collectives — collectives pay a ~77us start-skew
rendezvous here, local DMA does not):
  * Replicate the embedding table and the (fp16-cast, i/g/o-only, transposed)
    W_ih on every core; each core runs the full 4-layer single-step chain,
    streaming weights chunk-by-chunk under the matmuls.
  * Core c writes output[64c:64c+64] (t-sharded, contiguous rows) and the 16
    state slots [16c:16c+16] of state_h/state_c; host concatenates.
"""
import os
import sys

sys.path.insert(0, "/opt/trn_rl_repo")

import numpy as np
import concourse.bass as bass
import concourse.bacc as bacc
import concourse.tile as tile
import concourse.mybir as mybir
from concourse import bass_utils

B, S, H, L, V = 128, 512, 1024, 4, 32000
N_CORES = 8
TCHUNK = S // N_CORES  # 64 timesteps per core
JSLOT = B // N_CORES  # 16 state slots per core
G3 = 3 * H  # i|g|o packed gate width
KC = H // 128  # 8 contraction chunks
TS_PER_BUF = 4  # timesteps per output staging buffer (2MB DMAs)

F32 = mybir.dt.float32
F16 = mybir.dt.float16
I32 = mybir.dt.int32
ACTF = mybir.ActivationFunctionType
ALU = mybir.AluOpType


def _build():
    nc = bacc.Bacc("TRN2", target_bir_lowering=False, debug=False, num_devices=N_CORES)

    x32 = nc.dram_tensor("x32", [B, S], I32, kind="ExternalInput").ap()
    emb = nc.dram_tensor("emb", [V, H], F32, kind="ExternalInput").ap()
    w = nc.dram_tensor("w", [L, KC, 128, G3], F16, kind="ExternalInput").ap()
    bias = nc.dram_tensor("bias", [L, 1, G3], F16, kind="ExternalInput").ap()
    ident = nc.dram_tensor("ident", [128, 128], F16, kind="ExternalInput").ap()
    ltri = nc.dram_tensor("ltri", [128, 128], F32, kind="ExternalInput").ap()
    iota_t = nc.dram_tensor("iota_t", [128, TCHUNK], F32, kind="ExternalInput").ap()
    iota_j = nc.dram_tensor("iota_j", [128, JSLOT], F32, kind="ExternalInput").ap()

    out_chunk = nc.dram_tensor("out_chunk", [TCHUNK, B, H], F32, kind="ExternalOutput").ap()
    st_h = nc.dram_tensor("st_h", [L, JSLOT, H], F32, kind="ExternalOutput").ap()
    st_c = nc.dram_tensor("st_c", [L, JSLOT, H], F32, kind="ExternalOutput").ap()

    with tile.TileContext(nc) as tc:
        with (
            tc.tile_pool(name="consts", bufs=1) as cpool,
            tc.tile_pool(name="work", bufs=1) as wk,
            tc.tile_pool(name="hc", bufs=2) as hc,
            tc.tile_pool(name="weights", bufs=12) as wpool,
            tc.tile_pool(name="bias", bufs=2) as bpool,
            tc.tile_pool(name="xtT", bufs=2) as xpool,
            tc.tile_pool(name="obuf", bufs=2) as opool,
            tc.tile_pool(name="psum", bufs=1, space="PSUM") as pp,
            tc.tile_pool(name="psum_tr", bufs=2, space="PSUM") as ptr,
        ):
            # ---- constants -------------------------------------------------
            ident_sb = cpool.tile([128, 128], F16)
            nc.sync.dma_start(out=ident_sb[:], in_=ident[:])
            ltri_sb = cpool.tile([128, 128], F32)
            nc.sync.dma_start(out=ltri_sb[:], in_=ltri[:])
            iota_t_sb = cpool.tile([128, TCHUNK], F32)
            nc.sync.dma_start(out=iota_t_sb[:], in_=iota_t[:])
            iota_j_sb = cpool.tile([128, JSLOT], F32)
            nc.sync.dma_start(out=iota_j_sb[:], in_=iota_j[:])
            ones_sb = cpool.tile([1, 128], F16)
            nc.vector.memset(ones_sb[:], 1.0)

            # ---- lengths + selection matrix --------------------------------
            sx = wk.tile([B, S], I32)
            nc.sync.dma_start(out=sx[:], in_=x32[:])
            sxf = wk.tile([B, S], F32)
            nc.vector.tensor_copy(sxf[:], sx[:])
            nc.vector.tensor_scalar(
                out=sxf[:], in0=sxf[:], scalar1=0.0, scalar2=None, op0=ALU.is_gt
            )
            sm = wk.tile([B, 128], F32)  # smalls: lenf | e | rank | pt(3:19) | mask(64:)
            lenf = sm[:, 0:1]
            nc.vector.reduce_sum(out=lenf, in_=sxf[:], axis=mybir.AxisListType.X)
            e_sel = sm[:, 1:2]
            nc.vector.tensor_scalar(
                out=e_sel, in0=lenf, scalar1=float(S), scalar2=None, op0=ALU.is_equal
            )
            r_ps = ptr.tile([128, 1], F32, tag="tr")
            nc.tensor.matmul(r_ps[:], lhsT=ltri_sb[:], rhs=e_sel, start=True, stop=True)
            rank = sm[:, 2:3]
            nc.vector.tensor_scalar(
                out=rank, in0=r_ps[:], scalar1=-1.0, scalar2=None, op0=ALU.add
            )
            pt = sm[:, 3 : 3 + JSLOT]
            nc.vector.tensor_tensor(
                out=pt, in0=iota_j_sb[:], in1=rank.to_broadcast([B, JSLOT]),
                op=ALU.is_equal,
            )
            nc.vector.tensor_tensor(
                out=pt, in0=pt, in1=e_sel.to_broadcast([B, JSLOT]), op=ALU.mult
            )
            mask = sm[:, 64 : 64 + TCHUNK]
            nc.vector.tensor_tensor(
                out=mask, in0=iota_t_sb[:], in1=lenf.to_broadcast([B, TCHUNK]),
                op=ALU.is_lt,
            )

            # ---- embedding gather (t=0 tokens) -----------------------------
            xt0 = wk.tile([B, H], F32)
            nc.gpsimd.indirect_dma_start(
                out=xt0[:],
                out_offset=None,
                in_=emb[:],
                in_offset=bass.IndirectOffsetOnAxis(ap=sx[:, 0:1], axis=0),
            )

            def transpose_to_xtT(src_f16):
                """fp16 [128, H] -> fp16 transposed k-chunks [128, KC*128]."""
                dst = xpool.tile([128, KC * 128], F16, tag="xtT")
                for kc in range(KC):
                    tp = ptr.tile([128, 128], F16, tag="tr")
                    nc.tensor.transpose(
                        tp[:], src_f16[:, kc * 128 : (kc + 1) * 128], ident_sb[:]
                    )
                    nc.vector.tensor_copy(dst[:, kc * 128 : (kc + 1) * 128], tp[:])
                return dst

            xt0_16 = hc.tile([B, H], F16, tag="h16")
            nc.vector.tensor_copy(xt0_16[:], xt0[:])
            xtT = transpose_to_xtT(xt0_16)

            # ---- 4-layer single-step LSTM chain ----------------------------
            # gate slice order per k-chunk: i0,g0,o0,i1,g1,o1 so the s=0 ladder
            # can start while the s=1 slices still accumulate.
            slice_order = [0, 2, 4, 1, 3, 5]
            h_last = None
            for l in range(L):
                bias_sb = bpool.tile([1, G3], F16, tag="bias")
                nc.sync.dma_start(out=bias_sb[:], in_=bias[l])
                gates = pp.tile([128, G3], F32, tag="gates")
                for n in range(6):
                    nc.tensor.matmul(
                        gates[:, n * 512 : (n + 1) * 512],
                        lhsT=ones_sb[:],
                        rhs=bias_sb[0:1, n * 512 : (n + 1) * 512],
                        start=True,
                        stop=False,
                    )
                for kc in range(KC):
                    wt = wpool.tile([128, G3], F16, tag="w")
                    nc.sync.dma_start(out=wt[:], in_=w[l, kc])
                    for n in slice_order:
                        nc.tensor.matmul(
                            gates[:, n * 512 : (n + 1) * 512],
                            lhsT=xtT[:, kc * 128 : (kc + 1) * 128],
                            rhs=wt[:, n * 512 : (n + 1) * 512],
                            start=False,
                            stop=(kc == KC - 1),
                        )
                c_l = hc.tile([B, H], F32, tag="c_l")
                h_l = hc.tile([B, H], F32, tag="h_l")
                h16 = hc.tile([B, H], F16, tag="h16", name=f"h16_{l}") if l < L - 1 else None
                for s in range(2):
                    cols = slice(s * 512, s * 512 + 512)
                    sig_i = wk.tile([B, 512], F32, tag="sig_i", bufs=2)
                    nc.scalar.activation(
                        sig_i[:], gates[:, s * 512 : s * 512 + 512], ACTF.Sigmoid
                    )
                    tanh_g = wk.tile([B, 512], F32, tag="tanh_g", bufs=2)
                    nc.scalar.activation(
                        tanh_g[:], gates[:, H + s * 512 : H + s * 512 + 512], ACTF.Tanh
                    )
                    sig_o = wk.tile([B, 512], F32, tag="sig_o", bufs=2)
                    nc.scalar.activation(
                        sig_o[:],
                        gates[:, 2 * H + s * 512 : 2 * H + s * 512 + 512],
                        ACTF.Sigmoid,
                    )
                    nc.vector.tensor_tensor(
                        out=c_l[:, cols], in0=sig_i[:], in1=tanh_g[:], op=ALU.mult
                    )
                    tanh_c = wk.tile([B, 512], F32, tag="tanh_c", bufs=2)
                    nc.scalar.activation(tanh_c[:], c_l[:, cols], ACTF.Tanh)
                    nc.vector.tensor_tensor(
                        out=h_l[:, cols], in0=sig_o[:], in1=tanh_c[:], op=ALU.mult
                    )
                    if h16 is not None:
                        nc.vector.tensor_copy(h16[:, cols], h_l[:, cols])
                if h16 is not None:
                    xtT = transpose_to_xtT(h16)
                # states for this layer: select rows with length==S (stable order)
                for src, dst in ((h_l, st_h), (c_l, st_c)):
                    stile = wk.tile([JSLOT, H], F32, tag="stile", bufs=2, name=f"stile_{l}")
                    for n in range(2):
                        sp = ptr.tile([JSLOT, 512], F32, tag="tr", name=f"sp_{l}_{n}")
                        nc.tensor.matmul(
                            sp[:],
                            lhsT=pt,
                            rhs=src[:, n * 512 : (n + 1) * 512],
                            start=True,
                            stop=True,
                        )
                        nc.vector.tensor_copy(stile[:, n * 512 : (n + 1) * 512], sp[:])
                    nc.sync.dma_start(out=dst[l], in_=stile[:])
                h_last = h_l

            # ---- output: per-t masked broadcast of h4 ----------------------
            for g in range(TCHUNK // TS_PER_BUF):
                buf = opool.tile([128, TS_PER_BUF * H], F32, tag="obuf")
                for j in range(TS_PER_BUF):
                    t = g * TS_PER_BUF + j
                    mcol = sm[:, 64 + t : 65 + t]
                    dst = buf[:, j * H : (j + 1) * H]
                    if j % 2 == 0:
                        nc.vector.tensor_scalar(
                            out=dst, in0=h_last[:], scalar1=mcol, scalar2=None,
                            op0=ALU.mult,
                        )
                    else:
                        nc.scalar.activation(dst, h_last[:], ACTF.Copy, scale=mcol)
                nc.sync.dma_start(
                    out=out_chunk[g * TS_PER_BUF : (g + 1) * TS_PER_BUF].rearrange(
                        "t b h -> b t h"
                    ),
                    in_=buf[:],
                )


    nc.compile()
    return nc


_NC_CACHE = {}


def _get_nc():
    if "nc" not in _NC_CACHE:
        _NC_CACHE["nc"] = _build()
    return _NC_CACHE["nc"]


def _prep_in_maps(x, emb, W_ih, b_ih, b_hh):
    x32 = np.ascontiguousarray(np.asarray(x).astype(np.int32))
    emb = np.ascontiguousarray(np.asarray(emb, dtype=np.float32))
    W_ih = np.asarray(W_ih, dtype=np.float32)
    b_ih = np.asarray(b_ih, dtype=np.float32)
    b_hh = np.asarray(b_hh, dtype=np.float32)

    igo = np.r_[0:H, 2 * H : 3 * H, 3 * H : 4 * H]
    # rhs layout: [l, kc, k_part, j]  (j = i|g|o packed), fp16
    w_t = np.ascontiguousarray(
        W_ih[:, igo, :].transpose(0, 2, 1).reshape(L, KC, 128, G3).astype(np.float16)
    )
    bias = np.ascontiguousarray(
        (b_ih + b_hh)[:, igo].astype(np.float16).reshape(L, 1, G3)
    )
    ident = np.eye(128, dtype=np.float16)
    ltri = (np.arange(128)[:, None] <= np.arange(128)[None, :]).astype(np.float32)

    in_maps = []
    for c in range(N_CORES):
        iota_t = np.broadcast_to(
            (TCHUNK * c + np.arange(TCHUNK)).astype(np.float32), (128, TCHUNK)
        ).copy()
        iota_j = np.broadcast_to(
            (JSLOT * c + np.arange(JSLOT)).astype(np.float32), (128, JSLOT)
        ).copy()
        in_maps.append(
            {
                "x32": x32,
                "emb": emb,
                "w": w_t,
                "bias": bias,
                "ident": ident,
                "ltri": ltri,
                "iota_t": iota_t,
                "iota_j": iota_j,
            }
        )
    return in_maps


def kernel(x, emb, W_ih, W_hh, b_ih, b_hh):
    nc = _get_nc()
    in_maps = _prep_in_maps(x, emb, W_ih, b_ih, b_hh)
    trace = bool(os.environ.get("BASS_KERNEL_TRACE"))
    res = bass_utils.run_bass_kernel_spmd(
        nc, in_maps, core_ids=list(range(N_CORES)), trace=trace
    )
    if trace and res.exec_time_ns is not None:
        print(f"HW exec time: {res.exec_time_ns} ns")
    output = np.concatenate(
        [res.results[c]["out_chunk"] for c in range(N_CORES)], axis=0
    )
    state_h = np.concatenate(
        [res.results[c]["st_h"] for c in range(N_CORES)], axis=1
    )
    state_c = np.concatenate(
        [res.results[c]["st_c"] for c in range(N_CORES)], axis=1
    )
    return output, state_h, state_c
